# revision 2
# baseline (speedup 1.0000x reference)
"""Trainium2 Bass kernel v2 for nn_Discriminator (2-layer LSTM, B=512 T=100 H=1024).

Cross-core pipeline split (SPMD-uniform):
  - Cores 0-3 run LSTM layer 1 for batch slices of 128 rows each;
    cores 4-7 run layer 2 for the same slices, LAG=2 iterations behind,
    receiving h1^T per step via pairwise AllGather [[0,4],[1,5],[2,6],[3,7]].
  - Every matmul is 128-wide stationary (full PE array): per iteration each
    core streams 16 contraction chunks x 4096 gate cols in bf16.
  - Uniform instruction stream; roles differ only through per-core input
    data: weights (W_ih1/W_hh1 vs W_ih2/W_hh2), x (real vs zeros), and a
    vmask scalar that zeroes the gather-self-slot on layer-1 cores.
  - enc_W is folded into layer-1's input weights on the host
    (M1 = W_ih1 @ enc_W, contraction 34, zero-padded to chunk 0), so the
    v-side operand for layer 1 is just x_t^T. No A-phases, no DRAM staging,
    no on-device weight transposes (host pre-transposes everything).
"""

import numpy as np

import concourse.bass as bass
import concourse.tile as tile
import concourse.mybir as mybir
from concourse import bacc
from concourse.masks import make_identity

F32 = mybir.dt.float32
BF16 = mybir.dt.bfloat16
AF = mybir.ActivationFunctionType

N_CORES = 8
B, IN, H = 512, 34, 1024
G = 4 * H          # 4096
P = 128            # batch rows per core (and partition width)
KC = H // P        # 8 contraction chunks per operand
T = 100
LAG = 2            # iterations layer-2 cores trail layer-1 cores
GROUPS = [[0, 4], [1, 5], [2, 6], [3, 7]]

# slice s covers gate columns [s*512, (s+1)*512); gate order i,f,g,o
SLICE_ORDER = [0, 1, 4, 5, 2, 3, 6, 7]
SLICE_FUNC = {0: AF.Sigmoid, 1: AF.Sigmoid, 2: AF.Sigmoid, 3: AF.Sigmoid,
              4: AF.Tanh, 5: AF.Tanh, 6: AF.Sigmoid, 7: AF.Sigmoid}


def build(T_steps=T):
    nc = bacc.Bacc("TRN2", target_bir_lowering=False, debug=False,
                   num_devices=N_CORES)

    wvT_d = nc.dram_tensor("wvT", [P, KC, G], BF16, kind="ExternalInput").ap()
    whT_d = nc.dram_tensor("whT", [P, KC, G], BF16, kind="ExternalInput").ap()
    bias_d = nc.dram_tensor("bias128", [P, G], BF16, kind="ExternalInput").ap()
    xT_d = nc.dram_tensor("xpadT", [IN, T_steps * P], BF16, kind="ExternalInput").ap()
    vmask_d = nc.dram_tensor("vmask", [P, 1], F32, kind="ExternalInput").ap()
    decWT_d = nc.dram_tensor("decWT", [P, KC], BF16, kind="ExternalInput").ap()
    out = nc.dram_tensor("out", [P, 1], F32, kind="ExternalOutput").ap()

    send = nc.dram_tensor("send_scr", [2, P, KC * P], BF16).ap()
    gath = nc.dram_tensor("gath_scr", [2, 2, P, KC * P], BF16).ap()

    NIT = T_steps + LAG

    with tile.TileContext(nc) as tc:
        with tc.tile_pool(name="persist", bufs=1) as persist, \
             tc.tile_pool(name="gact", bufs=10) as gact, \
             tc.tile_pool(name="hpool", bufs=2) as hpool, \
             tc.tile_pool(name="psg", bufs=4, space="PSUM") as psg, \
             tc.tile_pool(name="pstr", bufs=2, space="PSUM") as pstr, \
             tc.tile_pool(name="pdec", bufs=1, space="PSUM") as pdec:

            identf = persist.tile([P, P], F32, tag="identf")
            make_identity(nc, identf[:])
            ident = persist.tile([P, P], BF16, tag="ident")
            nc.vector.tensor_copy(ident[:], identf[:])

            # iteration 0 needs bias+xT+wvT only (its h-side is skipped);
            # wvT is split across both HWDGE queues so it lands ~2x sooner,
            # and whT (first needed at iteration 1) streams behind it
            xT = persist.tile([IN, T_steps * P], BF16, tag="xT")
            nc.sync.dma_start(xT[:], xT_d)
            vmask = persist.tile([P, 1], F32, tag="vmask")
            nc.sync.dma_start(vmask[:], vmask_d)
            bias128 = persist.tile([P, G], BF16, tag="bias128")
            nc.sync.dma_start(bias128[:], bias_d)
            wvT = persist.tile([P, KC, G], BF16, tag="wvT")
            nc.sync.dma_start(wvT[:], wvT_d)
            decWT = persist.tile([P, KC], BF16, tag="decWT")
            nc.scalar.dma_start(decWT[:], decWT_d)
            whT = persist.tile([P, KC, G], BF16, tag="whT")
            # second HWDGE queue: streams in parallel with the sync-queue
            # loads above, so iteration 1's h-side starts sooner
            nc.scalar.dma_start(whT[:], whT_d)

            c_t = persist.tile([P, H], F32, tag="c_t")
            nc.gpsimd.memset(c_t[:], 0.0)
            hT = persist.tile([P, 2, KC, P], BF16, tag="hT")
            nc.gpsimd.memset(hT[:], 0.0)
            vT = persist.tile([P, 2, KC, P], BF16, tag="vT")
            nc.gpsimd.memset(vT[:], 0.0)
            recv = persist.tile([P, 2, KC * P], BF16, tag="recv")
            nc.gpsimd.memset(recv[:], 0.0)

            # vT(0): zeros + x block 0 into chunk 0 rows 0:IN
            nc.vector.tensor_add(vT[0:IN, 0, 0, :], vT[0:IN, 0, 0, :],
                                 xT[:, 0:P])

            with nc.named_scope("steps"):
                for i in range(NIT):
                    s_prev = (i + 1) % 2
                    s_cur = i % 2
                    acts = {}
                    tmp = {}
                    tanh_c = {}
                    h_bf = hpool.tile([P, H], BF16, tag="h_bf")

                    for si, s in enumerate(SLICE_ORDER):
                        pg = psg.tile([P, 512], F32, tag="pg")
                        sl = slice(s * 512, (s + 1) * 512)
                        # at i=0 the h state is all-zero: skip its matmuls
                        # preload the gate bias into PSUM (DVE, runs during
                        # the previous slice's matmul stream), then all
                        # matmuls accumulate onto it: no post-matmul bias
                        # add on the critical path into the activations
                        nc.vector.tensor_copy(pg[:], bias128[:, sl])
                        ops = ([(hT, whT, s_prev)] if i > 0 else []) \
                            + [(vT, wvT, s_cur)]
                        n_ops = len(ops) * KC
                        m = 0
                        for state, w, slot in ops:
                            for k in range(KC):
                                nc.tensor.matmul(pg[:], state[:, slot, k, :],
                                                 w[:, k, sl],
                                                 start=False,
                                                 stop=(m == n_ops - 1),
                                                 skip_group_check=True)
                                m += 1
                        at = gact.tile([P, 512], F32, tag="gact", name=f"a{s}")
                        hh = s % 2  # h/c half index for g,f,o slices
                        if s in (6, 7):
                            # o-gate: ACT+mul in 256-col quarters so the
                            # first h^T transposes can start ~0.6us earlier
                            for q in range(2):
                                qs = slice(q * 256, (q + 1) * 256)
                                nc.scalar.activation(at[:, qs], pg[:, qs],
                                                     SLICE_FUNC[s])
                                nc.vector.tensor_mul(
                                    h_bf[:, hh * 512 + q * 256:hh * 512 + (q + 1) * 256],
                                    at[:, qs], tanh_c[hh][:, qs])
                            acts[s] = at
                            continue
                        nc.scalar.activation(at[:], pg[:], SLICE_FUNC[s])
                        acts[s] = at
                        if s in (4, 5):  # tanh(g) ready -> tmp = sig(i)*tanh(g)
                            tt = gact.tile([P, 512], F32, tag="gact", name=f"t{hh}")
                            nc.vector.tensor_mul(tt[:], acts[s - 4][:], at[:])
                            tmp[hh] = tt
                        elif s in (2, 3):  # sig(f) ready -> c update + tanh(c)
                            ch = c_t[:, hh * 512:(hh + 1) * 512]
                            nc.vector.tensor_mul(ch, ch, at[:])
                            nc.vector.tensor_add(ch, ch, tmp[hh][:])
                            tc_ = gact.tile([P, 512], F32, tag="gact", name=f"c{hh}")
                            nc.scalar.activation(tc_[:], ch, AF.Tanh)
                            tanh_c[hh] = tc_


                        if s == 1 and i < NIT - 1:
                            # vT(i+1) = recv(i-1)*vmask + x block (DVE, off
                            # the PE critical path, early in the iteration)
                            j = min(i + 1, T_steps - 1)
                            nc.vector.tensor_scalar(
                                vT[:, s_prev, :, :].rearrange("p k b -> p (k b)"),
                                recv[:, s_prev, :], vmask[:], None,
                                mybir.AluOpType.mult)
                            nc.vector.tensor_add(vT[0:IN, s_prev, 0, :],
                                                 vT[0:IN, s_prev, 0, :],
                                                 xT[:, j * P:(j + 1) * P])

                    # h^T via PE transposes into the ring slot for this iter
                    for k in range(KC):
                        pt = pstr.tile([P, P], BF16, tag="ptr")
                        nc.tensor.transpose(pt[:], h_bf[:, k * P:(k + 1) * P],
                                            ident[:])
                        nc.vector.tensor_copy(hT[:, s_cur, k, :], pt[:])

                    # forward h^T to the partner core (pairwise AllGather);
                    # gather(i) is consumed at iteration i+LAG, so skip the
                    # never-consumed tail sends
                    if i < NIT - LAG:
                        nc.sync.dma_start(
                            send[s_cur],
                            hT[:, s_cur, :, :].rearrange("p k b -> p (k b)"))
                        nc.gpsimd.collective_compute(
                            "AllGather", mybir.AluOpType.bypass,
                            replica_groups=GROUPS,
                            ins=[send[s_cur]],
                            outs=[gath[s_cur]],
                        )
                        nc.sync.dma_start(recv[:, s_cur, :], gath[s_cur, 0])

                    if i == NIT - 1:
                        pd = pdec.tile([P, 1], F32, tag="pd")
                        for k in range(KC):
                            nc.tensor.matmul(pd[:], hT[:, s_cur, k, :],
                                             decWT[:, k:k + 1],
                                             start=(k == 0), stop=(k == KC - 1))
                        osb = gact.tile([P, 512], F32, tag="gact", name="osb")
                        nc.vector.tensor_copy(osb[:, 0:1], pd[:])
                        nc.sync.dma_start(out, osb[:, 0:1])

    nc.compile()
    return nc


def _prep_inputs(inputs, T_steps=T):
    """Build the 8 per-core input dicts from the full problem inputs."""
    import ml_dtypes
    bf = ml_dtypes.bfloat16
    f64 = np.float64

    def wT_layout(W):  # [G, H] -> [P, KC, G]
        return np.ascontiguousarray(
            W.T.reshape(KC, P, G).transpose(1, 0, 2)).astype(bf)

    x = np.asarray(inputs["x"], np.float32)
    enc_W = np.asarray(inputs["enc_W"], f64)
    enc_b = np.asarray(inputs["enc_b"], f64)
    W_ih1 = np.asarray(inputs["W_ih1"], f64)
    W_hh1 = np.asarray(inputs["W_hh1"], np.float32)
    W_ih2 = np.asarray(inputs["W_ih2"], np.float32)
    W_hh2 = np.asarray(inputs["W_hh2"], np.float32)
    dec_W = np.asarray(inputs["dec_W"], np.float32)

    M1 = (W_ih1 @ enc_W)  # [G, IN]
    M1pad = np.zeros((G, H), f64)
    M1pad[:, :IN] = M1
    bias1 = (enc_b @ W_ih1.T
             + np.asarray(inputs["b_ih1"], f64)
             + np.asarray(inputs["b_hh1"], f64)).astype(np.float32)
    bias2 = (np.asarray(inputs["b_ih2"], f64)
             + np.asarray(inputs["b_hh2"], f64)).astype(np.float32)

    wvT_1 = wT_layout(M1pad.astype(np.float32))
    whT_1 = wT_layout(W_hh1)
    wvT_2 = wT_layout(W_ih2)
    whT_2 = wT_layout(W_hh2)
    b128_1 = np.ascontiguousarray(
        np.broadcast_to(bias1[None, :], (P, G))).astype(bf)
    b128_2 = np.ascontiguousarray(
        np.broadcast_to(bias2[None, :], (P, G))).astype(bf)
    decWT = np.ascontiguousarray(dec_W[0].reshape(KC, P).T).astype(bf)
    xT_zero = np.zeros((IN, T_steps * P), bf)
    vm0 = np.zeros((P, 1), np.float32)
    vm1 = np.ones((P, 1), np.float32)

    in_maps = []
    for c in range(N_CORES):
        if c < 4:
            xs = x[c * P:(c + 1) * P, :T_steps]  # [P, T, IN]
            xT = np.ascontiguousarray(
                xs.transpose(2, 1, 0).reshape(IN, T_steps * P)).astype(bf)
            m = {"wvT": wvT_1, "whT": whT_1, "bias128": b128_1,
                 "xpadT": xT, "vmask": vm0, "decWT": decWT}
        else:
            m = {"wvT": wvT_2, "whT": whT_2, "bias128": b128_2,
                 "xpadT": xT_zero, "vmask": vm1, "decWT": decWT}
        in_maps.append(m)
    return in_maps


_cached_nc = None
_cached_fn = None


def _build_jitted(nc):
    """jit(shard_map(bass_exec)) over 8 cores; cached across kernel() calls."""
    import jax
    from jax.sharding import Mesh, PartitionSpec
    from jax.experimental.shard_map import shard_map
    from concourse import bass2jax, mybir as _mybir

    bass2jax.install_neuronx_cc_hook()
    partition_name = nc.partition_id_tensor.name if nc.partition_id_tensor else None
    in_names, out_names, out_avals, zero_outs = [], [], [], []
    for alloc in nc.m.functions[0].allocations:
        if not isinstance(alloc, _mybir.MemoryLocationSet):
            continue
        name = alloc.memorylocations[0].name
        if alloc.kind == "ExternalInput":
            if name != partition_name:
                in_names.append(name)
        elif alloc.kind == "ExternalOutput":
            shape = tuple(alloc.tensor_shape)
            dtype = _mybir.dt.np(alloc.dtype)
            out_names.append(name)
            out_avals.append(jax.core.ShapedArray(shape, dtype))
            zero_outs.append(np.zeros(shape, dtype))
    n_params = len(in_names)
    n_outs = len(out_avals)
    all_in_names = list(in_names) + list(out_names)
    if partition_name is not None:
        all_in_names.append(partition_name)
    donate = tuple(range(n_params, n_params + n_outs))

    def _body(*args):
        operands = list(args)
        if partition_name is not None:
            operands.append(bass2jax.partition_id_tensor())
        outs = bass2jax._bass_exec_p.bind(
            *operands,
            out_avals=tuple(out_avals),
            in_names=tuple(all_in_names),
            out_names=tuple(out_names),
            lowering_input_output_aliases=(),
            sim_require_finite=True,
            sim_require_nnan=True,
            nc=nc,
        )
        return tuple(outs)

    devices = jax.devices()[:N_CORES]
    mesh = Mesh(np.asarray(devices), ("core",))
    in_specs = (PartitionSpec("core"),) * (n_params + n_outs)
    out_specs = (PartitionSpec("core"),) * n_outs
    fn = jax.jit(
        shard_map(_body, mesh=mesh, in_specs=in_specs, out_specs=out_specs,
                  check_rep=False),
        donate_argnums=donate, keep_unused=True,
    )
    return fn, in_names, out_names, zero_outs


_dev_cache = {}


def _to_device(name, arr):
    import hashlib
    import jax
    d = hashlib.blake2b(np.ascontiguousarray(arr).tobytes(),
                        digest_size=16).digest()
    hit = _dev_cache.get(name)
    if hit is not None and hit[0] == d:
        return hit[1]
    darr = jax.device_put(arr)
    _dev_cache[name] = (d, darr)
    return darr


def kernel(**inputs):
    global _cached_nc, _cached_fn
    if _cached_nc is None:
        _cached_nc = build(T)
        _cached_fn = _build_jitted(_cached_nc)
    fn, in_names, out_names, zero_outs = _cached_fn

    in_maps = _prep_inputs(inputs)
    dec_b = float(np.asarray(inputs["dec_b"], np.float32)[0])

    concat_in = []
    for name in in_names:
        arr = np.concatenate([np.asarray(m[name]) for m in in_maps], axis=0)
        concat_in.append(_to_device(name, arr))
    i = out_names.index("out")
    last_err = None
    for attempt in range(3):
        try:
            concat_zeros = [np.zeros((N_CORES * z.shape[0], *z.shape[1:]), z.dtype)
                            for z in zero_outs]
            out_arrs = fn(*concat_in, *concat_zeros)
            full = np.asarray(out_arrs[i]).astype(np.float32)  # [8*P, 1]
            res = np.empty((B, 1), np.float32)
            for c in range(4):
                res[c * P:(c + 1) * P] = full[(4 + c) * P:(5 + c) * P]
            return res + dec_b
        except Exception as e:
            last_err = e
            _dev_cache.clear()
            concat_in = [_to_device(name, np.concatenate(
                [np.asarray(m[name]) for m in in_maps], axis=0))
                for name in in_names]
    raise last_err
